# revision 14
# baseline (speedup 1.0000x reference)
"""AxialAttention row-attention kernel for 8 TRN2 NeuronCores.

Sharding: the 64 folded (b*h) MSA rows are split 8-per-core; each row's
LayerNorm + row attention + gating + output projection is computed fully
on-core (no collectives). Weights are replicated.

Host-side preprocessing folds the LayerNorm affine (g, b) and the q-scale
into the projection weights/biases:
    xn = (x - mu) * rstd * g + b
    xn @ W = ((x - mu) * rstd) @ (g[:, None] * W) + b @ W
so the on-chip LN only computes (x - mu) * rstd, and each projection gets a
precomputed bias vector (b @ W), applied via rank-1 ones matmuls into PSUM
(skipped when the host sees the bias is exactly zero).

Layouts per row (w=512 tokens, d=256, inner=512, 8 heads x 64):
  x natural [tok, d] -> LN -> transpose (PE) -> xnT [d, tok]
  qT = Wq'.T @ xnT   [inner, tok]     kT = Wk'.T @ xnT   [inner, tok]
  v  = xnT.T @ Wv'   [tok, inner] (stored with a ones column per head)
  g  = xnT.T @ Wg'   [tok, inner] -> sigmoid via tanh (same ACT table set
       as exp: sig(x) = 0.5*tanh(x/2) + 0.5, affine done on DVE)
  dotsT_h = kT_h.T @ qT_h  [j, i] -> exp (no max-subtract: |dots| < ~8)
  U_h = expT_h.T @ [v_h | 1]  [i, 65]; col 64 = softmax denominator Z
  out_g = (U * (1/Z) bcast) * sig   (DVE, fused PSUM->SBUF)
  y = out_gT.T @ Wo + bo  [tok, d]

ACT table-set discipline: all LN Sqrt ops run in a stats prephase, then the
main loop uses only Exp/Tanh (one set) -> 2 ACT_TABLE_LOADs per kernel.
"""

import os
import sys

import numpy as np

sys.path.insert(0, "/opt/trn_rl_repo")

P = 128
W = 512  # tokens per row
D = 256  # model dim
INNER = 512
H = 8  # heads
DH = 64
ROWS = 64  # total folded rows
NCORES = 8
RPC = ROWS // NCORES  # rows per core
EPS = 1e-5

_cache = {}


def _build_nc(skip_qkv_bias, skip_bo):
    import concourse.bass as bass  # noqa: F401
    from concourse import bacc
    from concourse import mybir
    from concourse import tile
    from concourse.masks import make_identity
    from contextlib import ExitStack

    BF = mybir.dt.bfloat16
    F32 = mybir.dt.float32

    nc = bacc.Bacc()

    x_d = nc.declare_dram_parameter("x", [RPC, W, D], F32, isOutput=False)
    wq_d = nc.declare_dram_parameter("wq", [D, INNER], BF, isOutput=False)
    wk_d = nc.declare_dram_parameter("wk", [D, INNER], BF, isOutput=False)
    wv_d = nc.declare_dram_parameter("wv", [D, INNER], BF, isOutput=False)
    wg_d = nc.declare_dram_parameter("wg", [D, INNER], BF, isOutput=False)
    wo_d = nc.declare_dram_parameter("wo", [INNER, D], BF, isOutput=False)
    cq_d = nc.declare_dram_parameter("cq", [1, INNER], BF, isOutput=False)
    ck_d = nc.declare_dram_parameter("ck", [1, INNER], BF, isOutput=False)
    cv_d = nc.declare_dram_parameter("cv", [1, INNER], BF, isOutput=False)
    cg_d = nc.declare_dram_parameter("cg", [1, INNER], BF, isOutput=False)
    bo_d = nc.declare_dram_parameter("bo", [1, D], BF, isOutput=False)
    out_d = nc.declare_dram_parameter("out", [RPC, W, D], F32, isOutput=True)

    x_ap = x_d.ap()
    out_ap = out_d.ap()

    Exp = mybir.ActivationFunctionType.Exp
    Tanh = mybir.ActivationFunctionType.Tanh
    Sqrt = mybir.ActivationFunctionType.Sqrt
    SUB = mybir.AluOpType.subtract
    MULT = mybir.AluOpType.mult
    ADD = mybir.AluOpType.add

    with tile.TileContext(nc) as tc, ExitStack() as ctx:
        consts = ctx.enter_context(tc.tile_pool(name="consts", bufs=1))
        # weights: [d, inner] as [128, 2(kc), inner]
        wq_sb = consts.tile([P, 2, INNER], BF, tag="wq")
        wk_sb = consts.tile([P, 2, INNER], BF, tag="wk")
        wv_sb = consts.tile([P, 2, INNER], BF, tag="wv")
        wg_sb = consts.tile([P, 2, INNER], BF, tag="wg")
        wo_sb = consts.tile([P, 4, D], BF, tag="wo")
        for w_sb, w_d in ((wq_sb, wq_d), (wk_sb, wk_d), (wv_sb, wv_d), (wg_sb, wg_d)):
            nc.sync.dma_start(out=w_sb, in_=w_d.ap().rearrange("(k p) i -> p k i", p=P))
        nc.sync.dma_start(out=wo_sb, in_=wo_d.ap().rearrange("(k p) i -> p k i", p=P))
        cq_sb = consts.tile([1, INNER], BF, tag="cq")
        ck_sb = consts.tile([1, INNER], BF, tag="ck")
        cv_sb = consts.tile([1, INNER], BF, tag="cv")
        cg_sb = consts.tile([1, INNER], BF, tag="cg")
        bo_sb = consts.tile([1, D], BF, tag="bo")
        for c_sb, c_d in (
            (cq_sb, cq_d),
            (ck_sb, ck_d),
            (cv_sb, cv_d),
            (cg_sb, cg_d),
            (bo_sb, bo_d),
        ):
            nc.sync.dma_start(out=c_sb, in_=c_d.ap())
        ones_row = consts.tile([1, INNER], BF, tag="ones")
        nc.vector.memset(ones_row, 1.0)
        ident = consts.tile([P, P], BF, tag="ident")
        make_identity(nc, ident)
        eps_t = consts.tile([P, 1], F32, tag="eps")
        nc.vector.memset(eps_t, EPS)

        # working pools
        xpool = ctx.enter_context(tc.tile_pool(name="xpool", bufs=RPC))
        mspool = ctx.enter_context(tc.tile_pool(name="mspool", bufs=RPC))
        stpool = ctx.enter_context(tc.tile_pool(name="stpool", bufs=8))
        lnpool = ctx.enter_context(tc.tile_pool(name="lnpool", bufs=2))
        xnpool = ctx.enter_context(tc.tile_pool(name="xnpool", bufs=2))
        qkpool = ctx.enter_context(tc.tile_pool(name="qkpool", bufs=2))
        vpool = ctx.enter_context(tc.tile_pool(name="vpool", bufs=2))
        sigpool = ctx.enter_context(tc.tile_pool(name="sigpool", bufs=2))
        exppool = ctx.enter_context(tc.tile_pool(name="exppool", bufs=2))
        rpool = ctx.enter_context(tc.tile_pool(name="rpool", bufs=8))
        ogpool = ctx.enter_context(tc.tile_pool(name="ogpool", bufs=2))
        ypool = ctx.enter_context(tc.tile_pool(name="ypool", bufs=16))

        ps_big = ctx.enter_context(tc.tile_pool(name="ps_big", bufs=2, space="PSUM"))
        ps_tr = ctx.enter_context(tc.tile_pool(name="ps_tr", bufs=2, space="PSUM"))
        ps_u = ctx.enter_context(tc.tile_pool(name="ps_u", bufs=2, space="PSUM"))

        # ---- Phase 0: load all rows, compute LN stats (all ACT Sqrt together
        # so the sqrt table set is loaded exactly once)
        x_rows = []
        ms_rows = []
        for r in range(RPC):
            x_sb = xpool.tile([P, 4, D], F32, tag="x", name=f"x_sb{r}")
            nc.gpsimd.dma_start(out=x_sb, in_=x_ap[r].rearrange("(c p) d -> p c d", p=P))
            ms = mspool.tile([P, 4, 2], F32, tag="ms", name=f"ms{r}")
            for c in range(4):
                stats = stpool.tile([P, 6], F32, tag="stats", name="stats")
                nc.vector.bn_stats(out=stats, in_=x_sb[:, c, :])
                nc.vector.bn_aggr(out=ms[:, c, :], in_=stats)
                sd = stpool.tile([P, 1], F32, tag="sd", name="sd")
                nc.scalar.activation(
                    out=sd, in_=ms[:, c, 1:2], func=Sqrt, bias=eps_t, scale=1.0
                )
                nc.vector.reciprocal(out=ms[:, c, 1:2], in_=sd)
            x_rows.append(x_sb)
            ms_rows.append(ms)

        # ---- Phase 1: per-row pipeline (ACT uses only Exp/Tanh)
        for r in range(RPC):
            x_sb = x_rows[r]
            ms = ms_rows[r]

            # LN normalize -> bf16
            xt_bf = lnpool.tile([P, 4, D], BF, tag="xt")
            for c in range(4):
                nc.vector.tensor_scalar(
                    out=xt_bf[:, c, :],
                    in0=x_sb[:, c, :],
                    scalar1=ms[:, c, 0:1],
                    scalar2=ms[:, c, 1:2],
                    op0=SUB,
                    op1=MULT,
                )

            # transpose -> xnT [128, 2(kc), 512]
            xnT = xnpool.tile([P, 2, W], BF, tag="xnT")
            for c in range(4):
                for dc in range(2):
                    trp = ps_tr.tile([P, P], BF, tag="tr", name="trp")
                    nc.tensor.transpose(trp, xt_bf[:, c, dc * P : (dc + 1) * P], ident)
                    nc.any.tensor_copy(xnT[:, dc, c * P : (c + 1) * P], trp)

            # qT, kT projections: [128, 4(m), 512], paired PSUM + one copy
            qT = qkpool.tile([P, 4, W], BF, tag="qT")
            kT = qkpool.tile([P, 4, W], BF, tag="kT")
            for dst, wsb, csb in ((qT, wq_sb, cq_sb), (kT, wk_sb, ck_sb)):
                for t in range(2):
                    pp = ps_big.tile([P, 2, W], F32, tag="big", name="pp_qk")
                    for ii in range(2):
                        m = 2 * t + ii
                        for k in range(2):
                            nc.tensor.matmul(
                                pp[:, ii, :],
                                wsb[:, k, m * P : (m + 1) * P],
                                xnT[:, k, :],
                                start=(k == 0),
                                stop=(k == 1) and skip_qkv_bias,
                            )
                        if not skip_qkv_bias:
                            nc.tensor.matmul(
                                pp[:, ii, :],
                                csb[:, m * P : (m + 1) * P],
                                ones_row[:, :W],
                                start=False,
                                stop=True,
                            )
                    nc.any.tensor_copy(dst[:, 2 * t : 2 * t + 2, :], pp)

            # v natural with ones column: [128, 4(jc), 8(h), 65]
            v65 = vpool.tile([P, 4, H, 65], BF, tag="v65")
            nc.vector.memset(v65[:, :, :, 64:65], 1.0)
            for t in range(2):
                pp = ps_big.tile([P, 2, W], F32, tag="big", name="pp_v")
                for ii in range(2):
                    jc = 2 * t + ii
                    for k in range(2):
                        nc.tensor.matmul(
                            pp[:, ii, :],
                            xnT[:, k, jc * P : (jc + 1) * P],
                            wv_sb[:, k, :],
                            start=(k == 0),
                            stop=(k == 1) and skip_qkv_bias,
                        )
                    if not skip_qkv_bias:
                        nc.tensor.matmul(
                            pp[:, ii, :], ones_row[:, :P], cv_sb, start=False, stop=True
                        )
                nc.any.tensor_copy(
                    v65[:, 2 * t : 2 * t + 2, :, :64],
                    pp.rearrange("p a (h e) -> p a h e", e=DH),
                )

            # gates natural -> sig = 0.5*tanh((g+cg)/2) + 0.5: [128, 4(ic), 512]
            sig = sigpool.tile([P, 4, INNER], BF, tag="sig")
            for t in range(2):
                pp = ps_big.tile([P, 2, W], F32, tag="big", name="pp_g")
                for ii in range(2):
                    ic = 2 * t + ii
                    for k in range(2):
                        nc.tensor.matmul(
                            pp[:, ii, :],
                            xnT[:, k, ic * P : (ic + 1) * P],
                            wg_sb[:, k, :],
                            start=(k == 0),
                            stop=False,
                        )
                    nc.tensor.matmul(
                        pp[:, ii, :], ones_row[:, :P], cg_sb, start=False, stop=True
                    )
                nc.scalar.activation(
                    out=sig[:, 2 * t : 2 * t + 2, :], in_=pp, func=Tanh, scale=0.5
                )
            nc.vector.tensor_scalar(
                out=sig, in0=sig, scalar1=0.5, scalar2=0.5, op0=MULT, op1=ADD
            )

            # attention, head pairs (even head partitions 0:64, odd 64:128)
            out_g = ogpool.tile([P, 4, INNER], BF, tag="outg")
            for hp in range(4):
                expT = exppool.tile([P, 4, 2, W], BF, tag="expT")
                for jc in range(4):
                    dp = ps_big.tile([P, 2, W], F32, tag="big", name="dp")
                    for hpar in range(2):
                        poff = hpar * DH
                        nc.tensor.matmul(
                            dp[:, hpar, :],
                            kT[poff : poff + DH, hp, jc * P : (jc + 1) * P],
                            qT[poff : poff + DH, hp, :],
                            start=True,
                            stop=True,
                        )
                    nc.scalar.activation(out=expT[:, jc, :, :], in_=dp, func=Exp)
                for hpar in range(2):
                    h = 2 * hp + hpar
                    uph = ps_u.tile([P, 4, P], F32, tag="u", name="uph")
                    for ic in range(4):
                        for jc in range(4):
                            nc.tensor.matmul(
                                uph[:, ic, :65],
                                expT[:, jc, hpar, ic * P : (ic + 1) * P],
                                v65[:, jc, h, :],
                                start=(jc == 0),
                                stop=(jc == 3),
                            )
                    rt = rpool.tile([P, 4, 1], F32, tag="rt", name="rt")
                    nc.vector.reciprocal(out=rt, in_=uph[:, :, 64:65])
                    sigr = rpool.tile([P, 4, DH], BF, tag="sigr", name="sigr")
                    nc.vector.tensor_mul(
                        sigr,
                        sig[:, :, h * DH : (h + 1) * DH],
                        rt.to_broadcast([P, 4, DH]),
                    )
                    nc.vector.tensor_mul(
                        out_g[:, :, h * DH : (h + 1) * DH], uph[:, :, :64], sigr
                    )

            # transpose out_g -> out_gT [128, 4(kc), 512]
            ogT = ogpool.tile([P, 4, W], BF, tag="ogT")
            for ic in range(4):
                for k in range(4):
                    trp = ps_tr.tile([P, P], BF, tag="tr", name="trp2")
                    nc.tensor.transpose(trp, out_g[:, ic, k * P : (k + 1) * P], ident)
                    nc.any.tensor_copy(ogT[:, k, ic * P : (ic + 1) * P], trp)

            # final projection y = out_g @ Wo + bo, ic-pairs
            for t in range(2):
                yp = ps_big.tile([P, 2, W], F32, tag="big", name="yp")
                for ii in range(2):
                    ic = 2 * t + ii
                    for k in range(4):
                        nc.tensor.matmul(
                            yp[:, ii, :D],
                            ogT[:, k, ic * P : (ic + 1) * P],
                            wo_sb[:, k, :],
                            start=(k == 0),
                            stop=(k == 3) and skip_bo,
                        )
                    if not skip_bo:
                        nc.tensor.matmul(
                            yp[:, ii, :D], ones_row[:, :P], bo_sb, start=False, stop=True
                        )
                y_sb = ypool.tile([P, 2, D], F32, tag="y", name="y_sb")
                nc.any.tensor_copy(y_sb, yp[:, :, :D])
                nc.sync.dma_start(
                    out=out_ap[r, t * 2 * P : (t + 1) * 2 * P, :].rearrange(
                        "(a p) d -> p a d", p=P
                    ),
                    in_=y_sb,
                )

    # Walrus DIRECT2D DMA instructions accept a single sync wait, but Tile
    # sometimes attaches both the data-dependency wait and a DMA-lane
    # ordering wait (predecessor on the same rotating semaphore lane).
    # For this kernel the lane waits are redundant: every DMA source has a
    # single producer covered by the kept engine wait, x/y tiles use
    # dedicated slots (no reuse), DRAM store regions are disjoint, and no
    # consumer can observe a lane semaphore value early (each x lane carries
    # one DMA; y stores start only after the const loads' consumers ran).
    from concourse import mybir as _mybir

    for b in nc.m.functions[0].blocks:
        for i in b.instructions:
            if isinstance(i, _mybir.InstDMACopy) and len(i.sync_info.on_wait) > 1:
                keep = [
                    w
                    for w in i.sync_info.on_wait
                    if not w.ant_name.startswith(("DMAHW", "DMASW"))
                ]
                assert len(keep) == 1, (
                    i.name,
                    [(w.ant_name, w.wait_value) for w in i.sync_info.on_wait],
                )
                i.sync_info = _mybir.SyncInfo(
                    on_wait=keep, on_update=list(i.sync_info.on_update)
                )

    nc.finalize()
    return nc


def _prep_inputs(x, ln_g, ln_b, Wq, Wkv, Wg, bg, Wo, bo):
    import ml_dtypes

    bf = ml_dtypes.bfloat16
    x = np.asarray(x, np.float32)
    g = np.asarray(ln_g, np.float32)
    b = np.asarray(ln_b, np.float32)
    Wq = np.asarray(Wq, np.float32) * (DH**-0.5)
    Wkv = np.asarray(Wkv, np.float32)
    Wg = np.asarray(Wg, np.float32)
    bg = np.asarray(bg, np.float32)
    Wo = np.asarray(Wo, np.float32)
    bo = np.asarray(bo, np.float32)
    Wk, Wv = Wkv[:, :INNER], Wkv[:, INNER:]

    cq = b @ Wq
    ck = b @ Wk
    cv = b @ Wv
    cg = bg + b @ Wg
    skip_qkv_bias = not (np.any(cq) or np.any(ck) or np.any(cv))
    skip_bo = not np.any(bo)

    com = {
        "wq": (g[:, None] * Wq).astype(bf),
        "wk": (g[:, None] * Wk).astype(bf),
        "wv": (g[:, None] * Wv).astype(bf),
        "wg": (g[:, None] * Wg).astype(bf),
        "wo": Wo.astype(bf),
        "cq": cq[None, :].astype(bf),
        "ck": ck[None, :].astype(bf),
        "cv": cv[None, :].astype(bf),
        "cg": cg[None, :].astype(bf),
        "bo": bo[None, :].astype(bf),
    }
    xf = x.reshape(ROWS, W, D)
    in_maps = []
    for i in range(NCORES):
        m = dict(com)
        m["x"] = np.ascontiguousarray(xf[i * RPC : (i + 1) * RPC])
        in_maps.append(m)
    return in_maps, skip_qkv_bias, skip_bo


def _install_ntff_hook():
    """Shim antenv.axon_hooks (absent on this image) so trace=True works."""
    import types

    try:
        import antenv.axon_hooks  # noqa: F401

        return
    except ImportError:
        pass
    try:
        import antenv
        from trn_agent_boot.trn_boot import _ntff_profile_via_ctypes

        mod = types.ModuleType("antenv.axon_hooks")
        hook = [None]
        mod.set_axon_ntff_profile_hook = lambda h: hook.__setitem__(0, h)
        mod.get_axon_ntff_profile_hook = lambda: hook[0]
        antenv.axon_hooks = mod
        sys.modules["antenv.axon_hooks"] = mod
        mod.set_axon_ntff_profile_hook(
            _ntff_profile_via_ctypes("/opt/axon/libaxon_pjrt.so")
        )
        import concourse.bass_utils as bu

        bu.upload_artifacts = lambda tmpdir: f"local:{tmpdir}"
    except Exception as e:  # pragma: no cover
        print(f"ntff hook install failed: {e}", flush=True)


def kernel(x, ln_g, ln_b, Wq, Wkv, Wg, bg, Wo, bo):
    from concourse.bass_utils import run_bass_kernel_spmd

    in_maps, skip_qkv_bias, skip_bo = _prep_inputs(
        x, ln_g, ln_b, Wq, Wkv, Wg, bg, Wo, bo
    )
    key = ("nc", skip_qkv_bias, skip_bo)
    if key not in _cache:
        _cache[key] = _build_nc(skip_qkv_bias, skip_bo)
    nc = _cache[key]

    trace = bool(os.environ.get("KERNEL_TRACE"))
    if trace:
        _install_ntff_hook()
        tdir = os.environ.get("KERNEL_TRACE_DIR")
        if tdir:
            os.makedirs(tdir, exist_ok=True)
    res = run_bass_kernel_spmd(
        nc,
        in_maps,
        core_ids=list(range(NCORES)),
        trace=trace,
        tmpdir=os.environ.get("KERNEL_TRACE_DIR") or None,
    )
    if trace:
        print(f"HW exec time: {res.exec_time_ns} ns", flush=True)
        _cache["last_exec_time_ns"] = res.exec_time_ns
    out = np.empty((1, ROWS, W, D), np.float32)
    for i in range(NCORES):
        out[0, i * RPC : (i + 1) * RPC] = res.results[i]["out"]
    return out


# revision 15
# speedup vs baseline: 1.0384x; 1.0384x over previous
"""AxialAttention row-attention kernel for 8 TRN2 NeuronCores.

Sharding: the 64 folded (b*h) MSA rows are split 8-per-core; each row's
LayerNorm + row attention + gating + output projection is computed fully
on-core (no collectives). Weights are replicated.

Host-side preprocessing folds the LayerNorm affine (g, b) and the q-scale
into the projection weights/biases:
    xn = (x - mu) * rstd * g + b
    xn @ W = ((x - mu) * rstd) @ (g[:, None] * W) + b @ W
so the on-chip LN only computes (x - mu) * rstd, and each projection gets a
precomputed bias vector (b @ W), applied via rank-1 ones matmuls into PSUM
(skipped when the host sees the bias is exactly zero).

Layouts per row (w=512 tokens, d=256, inner=512, 8 heads x 64):
  x natural [tok, d] -> LN -> transpose (PE) -> xnT [d, tok]
  qT = Wq'.T @ xnT   [inner, tok]     kT = Wk'.T @ xnT   [inner, tok]
  v  = xnT.T @ Wv'   [tok, inner] (stored with a ones column per head)
  g  = xnT.T @ Wg'   [tok, inner] -> sigmoid via tanh (same ACT table set
       as exp: sig(x) = 0.5*tanh(x/2) + 0.5, affine done on DVE)
  dotsT_h = kT_h.T @ qT_h  [j, i] -> exp (no max-subtract: |dots| < ~8)
  U_h = expT_h.T @ [v_h | 1]  [i, 65]; col 64 = softmax denominator Z
  out_g = (U * (1/Z) bcast) * sig   (DVE, fused PSUM->SBUF)
  y = out_gT.T @ Wo + bo  [tok, d]

ACT table-set discipline: all LN Sqrt ops run in a stats prephase, then the
main loop uses only Exp/Tanh (one set) -> 2 ACT_TABLE_LOADs per kernel.
"""

import os
import sys

import numpy as np

sys.path.insert(0, "/opt/trn_rl_repo")

P = 128
W = 512  # tokens per row
D = 256  # model dim
INNER = 512
H = 8  # heads
DH = 64
ROWS = 64  # total folded rows
NCORES = 8
RPC = ROWS // NCORES  # rows per core
EPS = 1e-5

_cache = {}


def _build_nc(skip_qkv_bias, skip_bo):
    import concourse.bass as bass  # noqa: F401
    from concourse import bacc
    from concourse import mybir
    from concourse import tile
    from concourse.masks import make_identity
    from contextlib import ExitStack

    BF = mybir.dt.bfloat16
    F32 = mybir.dt.float32

    nc = bacc.Bacc()

    x_d = nc.declare_dram_parameter("x", [RPC, W, D], F32, isOutput=False)
    wq_d = nc.declare_dram_parameter("wq", [D, INNER], BF, isOutput=False)
    wk_d = nc.declare_dram_parameter("wk", [D, INNER], BF, isOutput=False)
    wv_d = nc.declare_dram_parameter("wv", [D, INNER], BF, isOutput=False)
    wg_d = nc.declare_dram_parameter("wg", [D, INNER], BF, isOutput=False)
    wo_d = nc.declare_dram_parameter("wo", [INNER, D], BF, isOutput=False)
    cq_d = nc.declare_dram_parameter("cq", [1, INNER], BF, isOutput=False)
    ck_d = nc.declare_dram_parameter("ck", [1, INNER], BF, isOutput=False)
    cv_d = nc.declare_dram_parameter("cv", [1, INNER], BF, isOutput=False)
    cg_d = nc.declare_dram_parameter("cg", [1, INNER], BF, isOutput=False)
    bo_d = nc.declare_dram_parameter("bo", [1, D], BF, isOutput=False)
    out_d = nc.declare_dram_parameter("out", [RPC, W, D], F32, isOutput=True)

    x_ap = x_d.ap()
    out_ap = out_d.ap()

    Exp = mybir.ActivationFunctionType.Exp
    Tanh = mybir.ActivationFunctionType.Tanh
    Sqrt = mybir.ActivationFunctionType.Sqrt
    SUB = mybir.AluOpType.subtract
    MULT = mybir.AluOpType.mult
    ADD = mybir.AluOpType.add

    with tile.TileContext(nc) as tc, ExitStack() as ctx:
        consts = ctx.enter_context(tc.tile_pool(name="consts", bufs=1))
        # weights: [d, inner] as [128, 2(kc), inner]
        wq_sb = consts.tile([P, 2, INNER], BF, tag="wq")
        wk_sb = consts.tile([P, 2, INNER], BF, tag="wk")
        wv_sb = consts.tile([P, 2, INNER], BF, tag="wv")
        wg_sb = consts.tile([P, 2, INNER], BF, tag="wg")
        wo_sb = consts.tile([P, 4, D], BF, tag="wo")
        for w_sb, w_d in ((wq_sb, wq_d), (wk_sb, wk_d), (wv_sb, wv_d), (wg_sb, wg_d)):
            nc.sync.dma_start(out=w_sb, in_=w_d.ap().rearrange("(k p) i -> p k i", p=P))
        nc.sync.dma_start(out=wo_sb, in_=wo_d.ap().rearrange("(k p) i -> p k i", p=P))
        cq_sb = consts.tile([1, INNER], BF, tag="cq")
        ck_sb = consts.tile([1, INNER], BF, tag="ck")
        cv_sb = consts.tile([1, INNER], BF, tag="cv")
        cg_sb = consts.tile([1, INNER], BF, tag="cg")
        bo_sb = consts.tile([1, D], BF, tag="bo")
        for c_sb, c_d in (
            (cq_sb, cq_d),
            (ck_sb, ck_d),
            (cv_sb, cv_d),
            (cg_sb, cg_d),
            (bo_sb, bo_d),
        ):
            nc.sync.dma_start(out=c_sb, in_=c_d.ap())
        ones_row = consts.tile([1, INNER], BF, tag="ones")
        nc.vector.memset(ones_row, 1.0)
        ident = consts.tile([P, P], BF, tag="ident")
        make_identity(nc, ident)
        eps_t = consts.tile([P, 1], F32, tag="eps")
        nc.vector.memset(eps_t, EPS)

        # working pools
        xpool = ctx.enter_context(tc.tile_pool(name="xpool", bufs=RPC))
        stpool = ctx.enter_context(tc.tile_pool(name="stpool", bufs=8))
        lnpool = ctx.enter_context(tc.tile_pool(name="lnpool", bufs=RPC))
        xnpool = ctx.enter_context(tc.tile_pool(name="xnpool", bufs=2))
        qkpool = ctx.enter_context(tc.tile_pool(name="qkpool", bufs=2))
        vpool = ctx.enter_context(tc.tile_pool(name="vpool", bufs=2))
        sigpool = ctx.enter_context(tc.tile_pool(name="sigpool", bufs=2))
        exppool = ctx.enter_context(tc.tile_pool(name="exppool", bufs=2))
        rpool = ctx.enter_context(tc.tile_pool(name="rpool", bufs=8))
        ogpool = ctx.enter_context(tc.tile_pool(name="ogpool", bufs=2))
        ypool = ctx.enter_context(tc.tile_pool(name="ypool", bufs=16))

        ps_big = ctx.enter_context(tc.tile_pool(name="ps_big", bufs=2, space="PSUM"))
        ps_tr = ctx.enter_context(tc.tile_pool(name="ps_tr", bufs=2, space="PSUM"))
        ps_u = ctx.enter_context(tc.tile_pool(name="ps_u", bufs=2, space="PSUM"))

        # ---- Phase 0: load rows, LN stats + normalize (all ACT Sqrt ops
        # grouped here so the sqrt table set loads exactly once; normalize
        # interleaved per row so row 0 compute starts immediately)
        xt_rows = []
        for r in range(RPC):
            x_sb = xpool.tile([P, 4, D], F32, tag="x", name=f"x_sb{r}")
            nc.gpsimd.dma_start(out=x_sb, in_=x_ap[r].rearrange("(c p) d -> p c d", p=P))
            xt_bf = lnpool.tile([P, 4, D], BF, tag="xt", name=f"xt{r}")
            for c in range(4):
                stats = stpool.tile([P, 6], F32, tag="stats", name="stats")
                nc.vector.bn_stats(out=stats, in_=x_sb[:, c, :])
                mv = stpool.tile([P, 2], F32, tag="mv", name="mv")
                nc.vector.bn_aggr(out=mv, in_=stats)
                sd = stpool.tile([P, 1], F32, tag="sd", name="sd")
                nc.scalar.activation(
                    out=sd, in_=mv[:, 1:2], func=Sqrt, bias=eps_t, scale=1.0
                )
                rstd = stpool.tile([P, 1], F32, tag="rstd", name="rstd")
                nc.vector.reciprocal(out=rstd, in_=sd)
                nc.vector.tensor_scalar(
                    out=xt_bf[:, c, :],
                    in0=x_sb[:, c, :],
                    scalar1=mv[:, 0:1],
                    scalar2=rstd,
                    op0=SUB,
                    op1=MULT,
                )
            xt_rows.append(xt_bf)

        # ---- Phase 1: per-row pipeline (ACT uses only Exp/Tanh)
        for r in range(RPC):
            xt_bf = xt_rows[r]

            # transpose -> xnT [128, 2(kc), 512]; 4 transposes per PSUM slab,
            # one batched copy each
            xnT = xnpool.tile([P, 2, W], BF, tag="xnT")
            for dc in range(2):
                trp = ps_tr.tile([P, 4, P], BF, tag="tr", name="trp")
                for c in range(4):
                    nc.tensor.transpose(
                        trp[:, c, :], xt_bf[:, c, dc * P : (dc + 1) * P], ident
                    )
                nc.any.tensor_copy(xnT[:, dc, :], trp)

            # qT, kT projections: [128, 4(m), 512], paired PSUM + one copy
            qT = qkpool.tile([P, 4, W], BF, tag="qT")
            kT = qkpool.tile([P, 4, W], BF, tag="kT")
            for dst, wsb, csb in ((qT, wq_sb, cq_sb), (kT, wk_sb, ck_sb)):
                for t in range(2):
                    pp = ps_big.tile([P, 2, W], F32, tag="big", name="pp_qk")
                    for ii in range(2):
                        m = 2 * t + ii
                        for k in range(2):
                            nc.tensor.matmul(
                                pp[:, ii, :],
                                wsb[:, k, m * P : (m + 1) * P],
                                xnT[:, k, :],
                                start=(k == 0),
                                stop=(k == 1) and skip_qkv_bias,
                            )
                        if not skip_qkv_bias:
                            nc.tensor.matmul(
                                pp[:, ii, :],
                                csb[:, m * P : (m + 1) * P],
                                ones_row[:, :W],
                                start=False,
                                stop=True,
                            )
                    nc.any.tensor_copy(dst[:, 2 * t : 2 * t + 2, :], pp)

            # v natural with ones column: [128, 4(jc), 8(h), 65]
            v65 = vpool.tile([P, 4, H, 65], BF, tag="v65")
            nc.vector.memset(v65[:, :, :, 64:65], 1.0)
            for t in range(2):
                pp = ps_big.tile([P, 2, W], F32, tag="big", name="pp_v")
                for ii in range(2):
                    jc = 2 * t + ii
                    for k in range(2):
                        nc.tensor.matmul(
                            pp[:, ii, :],
                            xnT[:, k, jc * P : (jc + 1) * P],
                            wv_sb[:, k, :],
                            start=(k == 0),
                            stop=(k == 1) and skip_qkv_bias,
                        )
                    if not skip_qkv_bias:
                        nc.tensor.matmul(
                            pp[:, ii, :], ones_row[:, :P], cv_sb, start=False, stop=True
                        )
                nc.any.tensor_copy(
                    v65[:, 2 * t : 2 * t + 2, :, :64],
                    pp.rearrange("p a (h e) -> p a h e", e=DH),
                )

            # gates natural -> sig = 0.5*tanh((g+cg)/2) + 0.5: [128, 4(ic), 512]
            sig = sigpool.tile([P, 4, INNER], BF, tag="sig")
            for t in range(2):
                pp = ps_big.tile([P, 2, W], F32, tag="big", name="pp_g")
                for ii in range(2):
                    ic = 2 * t + ii
                    for k in range(2):
                        nc.tensor.matmul(
                            pp[:, ii, :],
                            xnT[:, k, ic * P : (ic + 1) * P],
                            wg_sb[:, k, :],
                            start=(k == 0),
                            stop=False,
                        )
                    nc.tensor.matmul(
                        pp[:, ii, :], ones_row[:, :P], cg_sb, start=False, stop=True
                    )
                nc.scalar.activation(
                    out=sig[:, 2 * t : 2 * t + 2, :], in_=pp, func=Tanh, scale=0.5
                )
            nc.vector.tensor_scalar(
                out=sig, in0=sig, scalar1=0.5, scalar2=0.5, op0=MULT, op1=ADD
            )

            # attention, head pairs (even head partitions 0:64, odd 64:128)
            out_g = ogpool.tile([P, 4, INNER], BF, tag="outg")
            ogT = ogpool.tile([P, 4, W], BF, tag="ogT")
            for hp in range(4):
                expT = exppool.tile([P, 4, 2, W], BF, tag="expT")
                for jc in range(4):
                    dp = ps_big.tile([P, 2, W], F32, tag="big", name="dp")
                    for hpar in range(2):
                        poff = hpar * DH
                        nc.tensor.matmul(
                            dp[:, hpar, :],
                            kT[poff : poff + DH, hp, jc * P : (jc + 1) * P],
                            qT[poff : poff + DH, hp, :],
                            start=True,
                            stop=True,
                            tile_position=(poff, 0),
                        )
                    nc.scalar.activation(out=expT[:, jc, :, :], in_=dp, func=Exp)
                for hpar in range(2):
                    h = 2 * hp + hpar
                    uph = ps_u.tile([P, 4, P], F32, tag="u", name="uph")
                    for ic in range(4):
                        for jc in range(4):
                            nc.tensor.matmul(
                                uph[:, ic, :65],
                                expT[:, jc, hpar, ic * P : (ic + 1) * P],
                                v65[:, jc, h, :],
                                start=(jc == 0),
                                stop=(jc == 3),
                            )
                    rt = rpool.tile([P, 4, 1], F32, tag="rt", name="rt")
                    nc.vector.reciprocal(out=rt, in_=uph[:, :, 64:65])
                    sigr = rpool.tile([P, 4, DH], BF, tag="sigr", name="sigr")
                    nc.vector.tensor_mul(
                        sigr,
                        sig[:, :, h * DH : (h + 1) * DH],
                        rt.to_broadcast([P, 4, DH]),
                    )
                    nc.vector.tensor_mul(
                        out_g[:, :, h * DH : (h + 1) * DH], uph[:, :, :64], sigr
                    )

                # out_gT slice k=hp only needs heads 2hp, 2hp+1 -> transpose
                # this pair's columns now (overlaps next pair's dots/exp)
                trp2 = ps_tr.tile([P, 4, P], BF, tag="tr", name="trp2")
                for ic in range(4):
                    nc.tensor.transpose(
                        trp2[:, ic, :], out_g[:, ic, hp * P : (hp + 1) * P], ident
                    )
                nc.any.tensor_copy(ogT[:, hp, :], trp2)

            # final projection y = out_g @ Wo + bo, ic-pairs
            for t in range(2):
                yp = ps_big.tile([P, 2, W], F32, tag="big", name="yp")
                for ii in range(2):
                    ic = 2 * t + ii
                    for k in range(4):
                        nc.tensor.matmul(
                            yp[:, ii, :D],
                            ogT[:, k, ic * P : (ic + 1) * P],
                            wo_sb[:, k, :],
                            start=(k == 0),
                            stop=(k == 3) and skip_bo,
                        )
                    if not skip_bo:
                        nc.tensor.matmul(
                            yp[:, ii, :D], ones_row[:, :P], bo_sb, start=False, stop=True
                        )
                y_sb = ypool.tile([P, 2, D], F32, tag="y", name="y_sb")
                nc.any.tensor_copy(y_sb, yp[:, :, :D])
                nc.sync.dma_start(
                    out=out_ap[r, t * 2 * P : (t + 1) * 2 * P, :].rearrange(
                        "(a p) d -> p a d", p=P
                    ),
                    in_=y_sb,
                )

    # Walrus DIRECT2D DMA instructions accept a single sync wait, but Tile
    # sometimes attaches both the data-dependency wait and a DMA-lane
    # ordering wait (predecessor on the same rotating semaphore lane).
    # For this kernel the lane waits are redundant: every DMA source has a
    # single producer covered by the kept engine wait, x/y tiles use
    # dedicated slots (no reuse), DRAM store regions are disjoint, and no
    # consumer can observe a lane semaphore value early (each x lane carries
    # one DMA; y stores start only after the const loads' consumers ran).
    from concourse import mybir as _mybir

    for b in nc.m.functions[0].blocks:
        for i in b.instructions:
            if isinstance(i, _mybir.InstDMACopy) and len(i.sync_info.on_wait) > 1:
                keep = [
                    w
                    for w in i.sync_info.on_wait
                    if not w.ant_name.startswith(("DMAHW", "DMASW"))
                ]
                assert len(keep) == 1, (
                    i.name,
                    [(w.ant_name, w.wait_value) for w in i.sync_info.on_wait],
                )
                i.sync_info = _mybir.SyncInfo(
                    on_wait=keep, on_update=list(i.sync_info.on_update)
                )

    nc.finalize()
    return nc


def _prep_inputs(x, ln_g, ln_b, Wq, Wkv, Wg, bg, Wo, bo):
    import ml_dtypes

    bf = ml_dtypes.bfloat16
    x = np.asarray(x, np.float32)
    g = np.asarray(ln_g, np.float32)
    b = np.asarray(ln_b, np.float32)
    Wq = np.asarray(Wq, np.float32) * (DH**-0.5)
    Wkv = np.asarray(Wkv, np.float32)
    Wg = np.asarray(Wg, np.float32)
    bg = np.asarray(bg, np.float32)
    Wo = np.asarray(Wo, np.float32)
    bo = np.asarray(bo, np.float32)
    Wk, Wv = Wkv[:, :INNER], Wkv[:, INNER:]

    cq = b @ Wq
    ck = b @ Wk
    cv = b @ Wv
    cg = bg + b @ Wg
    skip_qkv_bias = not (np.any(cq) or np.any(ck) or np.any(cv))
    skip_bo = not np.any(bo)

    com = {
        "wq": (g[:, None] * Wq).astype(bf),
        "wk": (g[:, None] * Wk).astype(bf),
        "wv": (g[:, None] * Wv).astype(bf),
        "wg": (g[:, None] * Wg).astype(bf),
        "wo": Wo.astype(bf),
        "cq": cq[None, :].astype(bf),
        "ck": ck[None, :].astype(bf),
        "cv": cv[None, :].astype(bf),
        "cg": cg[None, :].astype(bf),
        "bo": bo[None, :].astype(bf),
    }
    xf = x.reshape(ROWS, W, D)
    in_maps = []
    for i in range(NCORES):
        m = dict(com)
        m["x"] = np.ascontiguousarray(xf[i * RPC : (i + 1) * RPC])
        in_maps.append(m)
    return in_maps, skip_qkv_bias, skip_bo


def _install_ntff_hook():
    """Shim antenv.axon_hooks (absent on this image) so trace=True works."""
    import types

    try:
        import antenv.axon_hooks  # noqa: F401

        return
    except ImportError:
        pass
    try:
        import antenv
        from trn_agent_boot.trn_boot import _ntff_profile_via_ctypes

        mod = types.ModuleType("antenv.axon_hooks")
        hook = [None]
        mod.set_axon_ntff_profile_hook = lambda h: hook.__setitem__(0, h)
        mod.get_axon_ntff_profile_hook = lambda: hook[0]
        antenv.axon_hooks = mod
        sys.modules["antenv.axon_hooks"] = mod
        mod.set_axon_ntff_profile_hook(
            _ntff_profile_via_ctypes("/opt/axon/libaxon_pjrt.so")
        )
        import concourse.bass_utils as bu

        bu.upload_artifacts = lambda tmpdir: f"local:{tmpdir}"
    except Exception as e:  # pragma: no cover
        print(f"ntff hook install failed: {e}", flush=True)


def kernel(x, ln_g, ln_b, Wq, Wkv, Wg, bg, Wo, bo):
    from concourse.bass_utils import run_bass_kernel_spmd

    in_maps, skip_qkv_bias, skip_bo = _prep_inputs(
        x, ln_g, ln_b, Wq, Wkv, Wg, bg, Wo, bo
    )
    key = ("nc", skip_qkv_bias, skip_bo)
    if key not in _cache:
        _cache[key] = _build_nc(skip_qkv_bias, skip_bo)
    nc = _cache[key]

    trace = bool(os.environ.get("KERNEL_TRACE"))
    if trace:
        _install_ntff_hook()
        tdir = os.environ.get("KERNEL_TRACE_DIR")
        if tdir:
            os.makedirs(tdir, exist_ok=True)
    res = run_bass_kernel_spmd(
        nc,
        in_maps,
        core_ids=list(range(NCORES)),
        trace=trace,
        tmpdir=os.environ.get("KERNEL_TRACE_DIR") or None,
    )
    if trace:
        print(f"HW exec time: {res.exec_time_ns} ns", flush=True)
        _cache["last_exec_time_ns"] = res.exec_time_ns
    out = np.empty((1, ROWS, W, D), np.float32)
    for i in range(NCORES):
        out[0, i * RPC : (i + 1) * RPC] = res.results[i]["out"]
    return out
